# revision 4
# baseline (speedup 1.0000x reference)
"""Multi-head attention (B=2, S=2048, DIM=512, H=8) on 8 Trainium2 cores.

Sharding: data-parallel over batch x tensor-parallel over heads.
Core c handles batch b = c // 4 and heads {2g, 2g+1} where g = c % 4
(i.e. output feature columns [128g : 128g+128]).  All sharding /
gathering happens host-side; no on-device collectives.

Per-core kernel (all fp32):
  - inputs arrive host-pretransposed as X^T [512, 2048] so the
    contraction dim lands on SBUF partitions,
  - Q^T, K^T projections produced in [out_dim(128), seq] layout
    (attention-ready: head h occupies partitions 64h..64h+63),
  - V produced in natural [seq, out_dim] layout with an extra ones
    column per head -> the ctx matmul also accumulates the softmax
    denominator for free,
  - scores^T = K_h @ Q_h^T per 128-row key tile, exp via ScalarE
    activation with the 1/sqrt(512) scale fused,
  - ctx^T accumulated over key tiles (moving dim = queries, N=512),
  - PE transpose back to natural layout, reciprocal + scale, DMA out.
"""

import os

import numpy as np

DIM = 512
NUM_HEADS = 8
D_HEAD = 64
B = 2
S = 2048
N_CORES = 8
P = 128  # partitions
NK = DIM // P  # 4 contraction tiles for projections
NT = S // P  # 16 key tiles
VSTRIDE = 132  # V tile stride: [h0(64) | ones | h1(64) | 3 pad]
SCALE = float(1.0 / np.sqrt(512.0))

_CACHE = {}


def _build_program():
    import concourse.tile as tile
    from concourse import bacc, mybir

    f32 = mybir.dt.float32
    nc = bacc.Bacc("TRN2", target_bir_lowering=False, debug=False)

    xqT = nc.dram_tensor("xqT", [DIM, S], f32, kind="ExternalInput").ap()
    xkT = nc.dram_tensor("xkT", [DIM, S], f32, kind="ExternalInput").ap()
    xvT = nc.dram_tensor("xvT", [DIM, S], f32, kind="ExternalInput").ap()
    wq = nc.dram_tensor("wq", [P, DIM], f32, kind="ExternalInput").ap()
    wk = nc.dram_tensor("wk", [P, DIM], f32, kind="ExternalInput").ap()
    wv = nc.dram_tensor("wv", [P, DIM], f32, kind="ExternalInput").ap()
    bq2 = nc.dram_tensor("bq2", [P, 1], f32, kind="ExternalInput").ap()
    bk2 = nc.dram_tensor("bk2", [P, 1], f32, kind="ExternalInput").ap()
    bvb = nc.dram_tensor("bvb", [P, P], f32, kind="ExternalInput").ap()
    ident = nc.dram_tensor("ident", [P, P], f32, kind="ExternalInput").ap()
    out = nc.dram_tensor("out", [S, P], f32, kind="ExternalOutput").ap()

    with tile.TileContext(nc) as tc:
        _emit(tc, mybir, xqT, xkT, xvT, wq, wk, wv, bq2, bk2, bvb, ident, out)
    nc.compile()
    return nc


def _emit(tc, mybir, xqT, xkT, xvT, wq, wk, wv, bq2, bk2, bvb, ident, out):
    from contextlib import ExitStack

    nc = tc.nc
    f32 = mybir.dt.float32
    Exp = mybir.ActivationFunctionType.Exp

    with ExitStack() as ctx:
        const = ctx.enter_context(tc.tile_pool(name="const", bufs=1))
        qk = ctx.enter_context(tc.tile_pool(name="qk", bufs=1))
        vpool = ctx.enter_context(tc.tile_pool(name="vpool", bufs=1))

        # constants
        wq_sb = const.tile([P, DIM], f32, tag="wq")
        wk_sb = const.tile([P, DIM], f32, tag="wk")
        wv_sb = const.tile([P, DIM], f32, tag="wv")
        bq_sb = const.tile([P, 1], f32, tag="bq")
        bk_sb = const.tile([P, 1], f32, tag="bk")
        bvb_sb = const.tile([P, P], f32, tag="bvb")
        id_sb = const.tile([P, P], f32, tag="ident")
        nc.sync.dma_start(wq_sb[:], wq[:])
        nc.sync.dma_start(wk_sb[:], wk[:])
        nc.sync.dma_start(wv_sb[:], wv[:])
        nc.sync.dma_start(bq_sb[:], bq2[:])
        nc.sync.dma_start(bk_sb[:], bk2[:])
        nc.sync.dma_start(bvb_sb[:], bvb[:])
        nc.sync.dma_start(id_sb[:], ident[:])

        # persistent projection outputs
        QT = qk.tile([P, S], f32, tag="QT")  # [out_dim, seq]
        KT = qk.tile([P, S], f32, tag="KT")
        V = vpool.tile([P, NT * VSTRIDE], f32, tag="V")  # 16 x [128, 132]

        with (
            tc.tile_pool(name="xin", bufs=8) as xin,
            tc.tile_pool(name="psproj", bufs=2, space="PSUM") as psproj,
            tc.tile_pool(name="psv", bufs=2, space="PSUM") as psv,
        ):
            # ---- Q^T / K^T projections: out[m, s] over 4 contraction tiles
            for name, xT, w_sb, b_sb, dst in (
                ("q", xqT, wq_sb, bq_sb, QT),
                ("k", xkT, wk_sb, bk_sb, KT),
            ):
                xt = []
                for k in range(NK):
                    t = xin.tile([P, S], f32, tag="xt")
                    nc.sync.dma_start(t[:], xT[k * P : (k + 1) * P, :])
                    xt.append(t)
                for c in range(4):
                    ps = psproj.tile([P, 512], f32, tag="ps")
                    for k in range(NK):
                        nc.tensor.matmul(
                            ps[:],
                            wq_sb[:, k * P : (k + 1) * P] if name == "q" else wk_sb[:, k * P : (k + 1) * P],
                            xt[k][:, c * 512 : (c + 1) * 512],
                            start=(k == 0),
                            stop=(k == NK - 1),
                        )
                    nc.vector.tensor_scalar_add(
                        dst[:, c * 512 : (c + 1) * 512], ps[:], b_sb[:, 0:1]
                    )

            # ---- V projection: natural layout [seq, out_dim] + ones column
            xvt = []
            for k in range(NK):
                t = xin.tile([P, S], f32, tag="xt")
                nc.sync.dma_start(t[:], xvT[k * P : (k + 1) * P, :])
                xvt.append(t)
            for ti in range(NT):
                ps = psv.tile([P, P], f32, tag="psv")
                for k in range(NK):
                    nc.tensor.matmul(
                        ps[:],
                        xvt[k][:, ti * P : (ti + 1) * P],
                        wv_sb[:, k * P : (k + 1) * P],
                        start=(k == 0),
                        stop=(k == NK - 1),
                    )
                o = ti * VSTRIDE
                nc.vector.memset(V[:, o + 64 : o + 65], 1.0)
                nc.vector.tensor_add(V[:, o : o + 64], ps[:, 0:64], bvb_sb[:, 0:64])
                nc.vector.tensor_add(
                    V[:, o + 65 : o + 129], ps[:, 64:128], bvb_sb[:, 64:128]
                )

        # ---- attention ----
        with (
            tc.tile_pool(name="pss", bufs=2, space="PSUM") as pss,
            tc.tile_pool(name="psc", bufs=1, space="PSUM") as psc,
            tc.tile_pool(name="pst", bufs=2, space="PSUM") as pst,
            tc.tile_pool(name="es", bufs=3) as espool,
            tc.tile_pool(name="csb", bufs=2) as csbpool,
            tc.tile_pool(name="osmall", bufs=4) as osmall,
        ):
            for h in range(2):
                hp = 64 * h
                for half in range(2):
                    q0 = 1024 * half
                    cps = psc.tile([65, 1024], f32, tag="cps")
                    for ti in range(NT):
                        sps = pss.tile([P, 1024], f32, tag="sps")
                        for cc in range(2):
                            nc.tensor.matmul(
                                sps[:, cc * 512 : (cc + 1) * 512],
                                KT[hp : hp + 64, ti * P : (ti + 1) * P],
                                QT[hp : hp + 64, q0 + cc * 512 : q0 + (cc + 1) * 512],
                                start=True,
                                stop=True,
                            )
                        es = espool.tile([P, 1024], f32, tag="es")
                        nc.scalar.activation(es[:], sps[:], Exp, scale=SCALE)
                        vo = ti * VSTRIDE + (64 if h == 1 else 0)
                        for cc in range(2):
                            nc.tensor.matmul(
                                cps[:, cc * 512 : (cc + 1) * 512],
                                V[:, vo : vo + 65],
                                es[:, cc * 512 : (cc + 1) * 512],
                                start=(ti == 0),
                                stop=(ti == NT - 1),
                            )
                    # evacuate ctx^T, transpose back, normalize, store
                    csb = csbpool.tile([65, 1024], f32, tag="csb")
                    nc.vector.tensor_copy(csb[:], cps[:])
                    sumcol = 64 if h == 0 else 0
                    for u in range(8):
                        tp = pst.tile([P, 65], f32, tag="tp")
                        nc.tensor.transpose(
                            tp[:], csb[:, u * P : (u + 1) * P], id_sb[0:65, 0:65]
                        )
                        r = osmall.tile([P, 1], f32, tag="recip")
                        nc.vector.reciprocal(r[:], tp[:, sumcol : sumcol + 1])
                        o = osmall.tile([P, 64], f32, tag="o")
                        if h == 0:
                            nc.vector.tensor_scalar_mul(o[:], tp[:, 0:64], r[:, 0:1])
                        else:
                            nc.vector.tensor_scalar_mul(o[:], tp[:, 1:65], r[:, 0:1])
                        nc.sync.dma_start(
                            out[q0 + u * P : q0 + (u + 1) * P, hp : hp + 64], o[:]
                        )


def _get_program():
    if "nc" not in _CACHE:
        _CACHE["nc"] = _build_program()
    return _CACHE["nc"]


def _shard_inputs(query, key, value, Wq, bq, Wk, bk, Wv, bv):
    """Build the 8 per-core input dicts."""
    ident = np.eye(P, dtype=np.float32)
    maps = []
    xT = {}
    for b in range(B):
        xT[b] = (
            np.ascontiguousarray(query[b].T),
            np.ascontiguousarray(key[b].T),
            np.ascontiguousarray(value[b].T),
        )

    def wslice(W, g):
        # want w[p, 128k + m] = W[128g + m, 128k + p]
        Ws = W[P * g : P * (g + 1), :]  # [m, 512]
        return np.ascontiguousarray(
            Ws.reshape(P, NK, P).transpose(2, 1, 0).reshape(P, DIM)
        )

    for c in range(N_CORES):
        b, g = c // 4, c % 4
        sl = slice(P * g, P * (g + 1))
        maps.append(
            {
                "xqT": xT[b][0],
                "xkT": xT[b][1],
                "xvT": xT[b][2],
                "wq": wslice(Wq, g),
                "wk": wslice(Wk, g),
                "wv": wslice(Wv, g),
                "bq2": np.ascontiguousarray(bq[sl].reshape(P, 1)),
                "bk2": np.ascontiguousarray(bk[sl].reshape(P, 1)),
                "bvb": np.ascontiguousarray(np.broadcast_to(bv[sl], (P, P))),
                "ident": ident,
            }
        )
    return maps


def _numpy_reference(query, key, value, mask, Wq, bq, Wk, bk, Wv, bv):
    """Pure-numpy fallback (only used when the mask isn't all ones)."""
    out = np.empty((B, S, DIM), dtype=np.float32)
    for b in range(B):
        q = (query[b] @ Wq.T + bq).reshape(S, NUM_HEADS, D_HEAD)
        k = (key[b] @ Wk.T + bk).reshape(S, NUM_HEADS, D_HEAD)
        v = (value[b] @ Wv.T + bv).reshape(S, NUM_HEADS, D_HEAD)
        for h in range(NUM_HEADS):
            s = q[:, h, :] @ k[:, h, :].T
            s = np.where(mask[b], s, np.float32(-10000.0))
            s = s / np.float32(np.sqrt(DIM))
            s = s - s.max(axis=-1, keepdims=True)
            e = np.exp(s)
            p = e / e.sum(axis=-1, keepdims=True)
            out[b, :, h * D_HEAD : (h + 1) * D_HEAD] = p @ v[:, h, :]
    return out


LAST_EXEC_NS = None
LAST_RESULTS = None


def kernel(query, key, value, mask, Wq, bq, Wk, bk, Wv, bv):
    global LAST_EXEC_NS, LAST_RESULTS
    query = np.asarray(query, dtype=np.float32)
    key = np.asarray(key, dtype=np.float32)
    value = np.asarray(value, dtype=np.float32)
    mask = np.asarray(mask)
    Wq = np.asarray(Wq, dtype=np.float32)
    bq = np.asarray(bq, dtype=np.float32)
    Wk = np.asarray(Wk, dtype=np.float32)
    bk = np.asarray(bk, dtype=np.float32)
    Wv = np.asarray(Wv, dtype=np.float32)
    bv = np.asarray(bv, dtype=np.float32)

    if not mask.all():
        return _numpy_reference(query, key, value, mask, Wq, bq, Wk, bk, Wv, bv)

    from concourse.bass_utils import run_bass_kernel_spmd

    nc = _get_program()
    in_maps = _shard_inputs(query, key, value, Wq, bq, Wk, bk, Wv, bv)
    trace = os.environ.get("KERNEL_TRACE", "0") == "1"
    tmpdir = os.environ.get("KERNEL_TRACE_DIR") or None
    try:
        res = run_bass_kernel_spmd(
            nc, in_maps, list(range(N_CORES)), trace=trace, tmpdir=tmpdir
        )
    except Exception:
        if not trace:
            raise
        res = run_bass_kernel_spmd(nc, in_maps, list(range(N_CORES)), trace=False)
    LAST_EXEC_NS = res.exec_time_ns
    LAST_RESULTS = res
    out = np.empty((B, S, DIM), dtype=np.float32)
    for c in range(N_CORES):
        b, g = c // 4, c % 4
        out[b, :, P * g : P * (g + 1)] = res.results[c]["out"]
    return out


# revision 19
# speedup vs baseline: 1.2853x; 1.2853x over previous
"""Multi-head attention (B=2, S=2048, DIM=512, H=8) on 8 Trainium2 cores.

Sharding: data-parallel over batch x tensor-parallel over heads.
Core c handles batch b = c // 4 and heads {2g, 2g+1} where g = c % 4
(i.e. output feature columns [128g : 128g+128]).  All sharding /
gathering happens host-side; no on-device collectives.

Per-core kernel, all matmuls in f32r (TF32) mode - single PE pass but
K<=64 per matmul, and hi/lo 64-row halves must not share a PSUM
accumulation group:
  - the host interleaves the contraction dim so the projections' eight
    K=64 matmuls all read partitions 0..63 and accumulate into one
    PSUM tile (base-0-only groups are safe),
  - Q^T, K^T, V^T projections all produced in [out_dim(128), seq]
    layout (N=512 moving dim, full f32r rate); V is then PE-transposed
    into natural [seq, out_dim] tiles with a ones column per head so
    the ctx matmul also accumulates the softmax denominator,
  - scores^T per 128-row key tile (K=64 per head), exp via ScalarE
    activation with the 1/sqrt(512) scale fused ([128,1024] tiles,
    f32r output),
  - ctx^T accumulated over key tiles as two K=64 halves into separate
    PSUM tiles (cA: keys 0-63 of the tile, cB: keys 64-127), joined by
    one DVE add per [65,1024] block,
  - PE transpose back to natural layout, reciprocal + scale, DMA out.

f32r producers are restricted: only DMA (from f32r DRAM), DVE copies
and ScalarE activations may write f32r tiles; host pre-rounds all
DMA-fed matmul operands to the TF32 grid (round-to-nearest-even).
"""

import os

import numpy as np

DIM = 512
NUM_HEADS = 8
D_HEAD = 64
B = 2
S = 2048
N_CORES = 8
P = 128  # partitions
NK = DIM // P  # 4 x-tiles for projections (each holds two 64-halves)
NT = S // P  # 16 key tiles
VSTRIDE = 132  # V tile stride: [h0(64) | ones | h1(64) | 3 pad]
SCALE = float(1.0 / np.sqrt(512.0))
CH = 512  # input DMA / projection chunk (columns of seq)

_CACHE = {}


def _build_program(has_bias):
    import concourse.tile as tile
    from concourse import bacc, mybir

    f32 = mybir.dt.float32
    f32r = mybir.dt.float32r
    nc = bacc.Bacc("TRN2", target_bir_lowering=False, debug=False)

    t = {}
    t["xqT"] = nc.dram_tensor("xqT", [DIM, S], f32r, kind="ExternalInput").ap()
    t["xkT"] = nc.dram_tensor("xkT", [DIM, S], f32r, kind="ExternalInput").ap()
    t["xvT"] = nc.dram_tensor("xvT", [DIM, S], f32r, kind="ExternalInput").ap()
    t["wq"] = nc.dram_tensor("wq", [64, 1024], f32r, kind="ExternalInput").ap()
    t["wk"] = nc.dram_tensor("wk", [64, 1024], f32r, kind="ExternalInput").ap()
    t["wv"] = nc.dram_tensor("wv", [64, 1024], f32r, kind="ExternalInput").ap()
    t["onescol"] = nc.dram_tensor("onescol", [P, 1], f32r, kind="ExternalInput").ap()
    t["identr"] = nc.dram_tensor("identr", [P, P], f32r, kind="ExternalInput").ap()
    t["ident"] = nc.dram_tensor("ident", [P, P], f32, kind="ExternalInput").ap()
    if has_bias:
        t["bq2"] = nc.dram_tensor("bq2", [P, 1], f32, kind="ExternalInput").ap()
        t["bk2"] = nc.dram_tensor("bk2", [P, 1], f32, kind="ExternalInput").ap()
        t["bvb"] = nc.dram_tensor("bvb", [P, P], f32, kind="ExternalInput").ap()
    t["out"] = nc.dram_tensor("out", [S, P], f32, kind="ExternalOutput").ap()

    with tile.TileContext(nc) as tc:
        _emit(tc, mybir, t, has_bias)
    nc.compile()
    return nc


def _emit(tc, mybir, io, has_bias):
    from contextlib import ExitStack

    nc = tc.nc
    f32 = mybir.dt.float32
    f32r = mybir.dt.float32r
    Exp = mybir.ActivationFunctionType.Exp

    mm = nc.tensor.matmul

    with ExitStack() as ctx:
        const = ctx.enter_context(tc.tile_pool(name="const", bufs=1))
        qk = ctx.enter_context(tc.tile_pool(name="qk", bufs=1))
        vpool = ctx.enter_context(tc.tile_pool(name="vpool", bufs=1))
        csbpool = ctx.enter_context(tc.tile_pool(name="csb", bufs=1))
        osmall = ctx.enter_context(tc.tile_pool(name="osmall", bufs=4))

        # constants
        wsb = {}
        for name in ("wq", "wk", "wv"):
            wsb[name] = const.tile([64, 1024], f32r, name=name, tag=name)
            nc.sync.dma_start(wsb[name][:], io[name][:])
        ones_sb = const.tile([P, 1], f32r, tag="onescol")
        nc.sync.dma_start(ones_sb[:], io["onescol"][:])
        idr_sb = const.tile([P, P], f32r, tag="identr")
        nc.sync.dma_start(idr_sb[:], io["identr"][:])
        id_sb = const.tile([P, P], f32, tag="ident")
        nc.sync.dma_start(id_sb[:], io["ident"][:])
        if has_bias:
            bq_sb = const.tile([P, 1], f32, tag="bq")
            bk_sb = const.tile([P, 1], f32, tag="bk")
            bvb_sb = const.tile([P, P], f32, tag="bvb")
            nc.sync.dma_start(bq_sb[:], io["bq2"][:])
            nc.sync.dma_start(bk_sb[:], io["bk2"][:])
            nc.sync.dma_start(bvb_sb[:], io["bvb"][:])

        # persistent projection outputs: [out_dim, seq], head h at
        # partitions 64h..64h+63
        QT = qk.tile([P, S], f32r, tag="QT")
        KT = qk.tile([P, S], f32r, tag="KT")
        V = vpool.tile([P, NT * VSTRIDE], f32r, tag="V")  # 16 x [128, 132]

        with (
            tc.tile_pool(name="xin", bufs=24) as xin,
            tc.tile_pool(name="psq", bufs=2, space="PSUM") as psq,
            tc.tile_pool(name="pstv", bufs=2, space="PSUM") as pstv,
            tc.tile_pool(name="ptmp", bufs=2) as ptmp,
        ):
            for c in range(S // CH):
                cs = slice(c * CH, (c + 1) * CH)
                xt = {}
                for name, key in (("q", "xqT"), ("k", "xkT"), ("v", "xvT")):
                    ts = []
                    for k in range(NK):
                        # [64, 2*CH]: the two 64-row halves of x-tile k
                        # side by side, so all matmuls read base 0
                        tl = xin.tile([64, 2 * CH], f32r, tag="xt")
                        nc.sync.dma_start(
                            tl[:].rearrange("p (j s) -> p j s", j=2),
                            io[key][k * P : (k + 1) * P, cs].rearrange(
                                "(j p) s -> p j s", p=64
                            ),
                        )
                        ts.append(tl)
                    xt[name] = ts

                # Q^T / K^T / V^T chunks: 8 base-0 K=64 matmuls, one PSUM
                for name, w, dst in (("q", "wq", QT), ("k", "wk", KT)):
                    ps = psq.tile([P, CH], f32, tag="psq")
                    for k in range(NK):
                        for j in range(2):
                            kk = 2 * k + j
                            mm(
                                ps[:],
                                wsb[w][:, kk * P : (kk + 1) * P],
                                xt[name][k][:, j * CH : (j + 1) * CH],
                                start=(kk == 0),
                                stop=(kk == 2 * NK - 1),
                            )
                    if has_bias:
                        b_sb = bq_sb if name == "q" else bk_sb
                        tmp = ptmp.tile([P, CH], f32, tag="ptmp")
                        nc.vector.tensor_scalar_add(tmp[:], ps[:], b_sb[:, 0:1])
                        nc.vector.tensor_copy(dst[:, cs], tmp[:])
                    else:
                        nc.vector.tensor_copy(dst[:, cs], ps[:])

                # V^T chunk, then transpose into natural V tiles
                ps = psq.tile([P, CH], f32, tag="psq")
                for k in range(NK):
                    for j in range(2):
                        kk = 2 * k + j
                        mm(
                            ps[:],
                            wsb["wv"][:, kk * P : (kk + 1) * P],
                            xt["v"][k][:, j * CH : (j + 1) * CH],
                            start=(kk == 0),
                            stop=(kk == 2 * NK - 1),
                        )
                vt = ptmp.tile([P, CH], f32r, tag="vtc")
                nc.vector.tensor_copy(vt[:], ps[:])
                for tl in range(CH // P):
                    ti = c * (CH // P) + tl
                    pv = pstv.tile([P, P], f32r, tag="pstv")
                    nc.tensor.transpose(
                        pv[:], vt[:, tl * P : (tl + 1) * P], idr_sb[:]
                    )
                    o = ti * VSTRIDE
                    nc.vector.tensor_copy(V[:, o + 64 : o + 65], ones_sb[:])
                    if has_bias:
                        tmpv = ptmp.tile([P, P], f32, tag="tmpv")
                        nc.vector.tensor_add(tmpv[:], pv[:], bvb_sb[:])
                        nc.vector.tensor_copy(V[:, o : o + 64], tmpv[:, 0:64])
                        nc.vector.tensor_copy(
                            V[:, o + 65 : o + 129], tmpv[:, 64:128]
                        )
                    else:
                        nc.vector.tensor_copy(V[:, o : o + 64], pv[:, 0:64])
                        nc.vector.tensor_copy(
                            V[:, o + 65 : o + 129], pv[:, 64:128]
                        )

        # ---- attention ----
        csbs = {}
        with (
            tc.tile_pool(name="pss", bufs=2, space="PSUM") as pss,
            tc.tile_pool(name="psc", bufs=1, space="PSUM") as psc,
            tc.tile_pool(name="es", bufs=3) as espool,
        ):
            for h in range(2):
                hp = 64 * h
                for half in range(2):
                    q0 = 1024 * half
                    cA = psc.tile([65, 1024], f32, tag="cA")
                    cB = psc.tile([65, 1024], f32, tag="cB")
                    for t in range(NT):
                        sps = pss.tile([P, 1024], f32, tag="sps")
                        for cc in range(2):
                            mm(
                                sps[:, cc * 512 : (cc + 1) * 512],
                                KT[hp : hp + 64, t * P : (t + 1) * P],
                                QT[hp : hp + 64, q0 + cc * 512 : q0 + (cc + 1) * 512],
                                start=True,
                                stop=True,
                            )
                        es = espool.tile([P, 1024], f32r, tag="es")
                        nc.scalar.activation(es[:], sps[:], Exp, scale=SCALE)
                        vo = t * VSTRIDE + (64 if h == 1 else 0)
                        for cc in range(2):
                            ccs = slice(cc * 512, (cc + 1) * 512)
                            mm(
                                cA[:, ccs],
                                V[0:64, vo : vo + 65],
                                es[0:64, ccs],
                                start=(t == 0),
                                stop=(t == NT - 1),
                            )
                            mm(
                                cB[:, ccs],
                                V[64:128, vo : vo + 65],
                                es[64:128, ccs],
                                start=(t == 0),
                                stop=(t == NT - 1),
                            )
                    csb = csbpool.tile([65, 1024], f32, tag=f"csb{h}{half}")
                    nc.vector.tensor_copy(csb[:], cA[:])
                    nc.vector.tensor_add(csb[:], csb[:], cB[:])
                    csbs[(h, half)] = csb

        # ---- transpose back to natural layout, normalize, store ----
        with tc.tile_pool(name="pst", bufs=2, space="PSUM") as pst:
            for h in range(2):
                hp = 64 * h
                sumcol = 64 if h == 0 else 0
                for half in range(2):
                    q0 = 1024 * half
                    csb = csbs[(h, half)]
                    for u in range(8):
                        tp = pst.tile([P, 65], f32, tag="tp")
                        nc.tensor.transpose(
                            tp[:], csb[:, u * P : (u + 1) * P], id_sb[0:65, 0:65]
                        )
                        r = osmall.tile([P, 1], f32, tag="recip")
                        nc.vector.reciprocal(r[:], tp[:, sumcol : sumcol + 1])
                        o = osmall.tile([P, 64], f32, tag="o")
                        if h == 0:
                            nc.vector.tensor_scalar_mul(o[:], tp[:, 0:64], r[:, 0:1])
                        else:
                            nc.vector.tensor_scalar_mul(o[:], tp[:, 1:65], r[:, 0:1])
                        nc.sync.dma_start(
                            io["out"][q0 + u * P : q0 + (u + 1) * P, hp : hp + 64],
                            o[:],
                        )


def _get_program(has_bias):
    key = ("nc", has_bias)
    if key not in _CACHE:
        _CACHE[key] = _build_program(has_bias)
    return _CACHE[key]


def _round_tf32(x):
    # round-to-nearest-even into the TF32 (10-bit mantissa) grid
    u = np.ascontiguousarray(x, dtype=np.float32).view(np.uint32)
    lsb = (u >> np.uint32(13)) & np.uint32(1)
    u = (u + np.uint32(0x0FFF) + lsb) & np.uint32(0xFFFFE000)
    return u.view(np.float32)


def _shard_inputs(query, key, value, Wq, bq, Wk, bk, Wv, bv, has_bias):
    """Build the 8 per-core input dicts."""
    ident = np.eye(P, dtype=np.float32)
    maps = []
    xT = {}
    for b in range(B):
        xT[b] = (
            _round_tf32(query[b].T),
            _round_tf32(key[b].T),
            _round_tf32(value[b].T),
        )

    def wslice(W, g):
        # want w[p, 128*kk + m] = W[128g + m, 64*kk + p]
        Ws = W[P * g : P * (g + 1), :]  # [m, 512]
        return _round_tf32(Ws.reshape(P, 8, 64).transpose(2, 1, 0).reshape(64, 1024))

    for c in range(N_CORES):
        b, g = c // 4, c % 4
        sl = slice(P * g, P * (g + 1))
        m = {
            "xqT": xT[b][0],
            "xkT": xT[b][1],
            "xvT": xT[b][2],
            "wq": wslice(Wq, g),
            "wk": wslice(Wk, g),
            "wv": wslice(Wv, g),
            "onescol": np.ones((P, 1), dtype=np.float32),
            "identr": ident,
            "ident": ident,
        }
        if has_bias:
            m["bq2"] = np.ascontiguousarray(bq[sl].reshape(P, 1))
            m["bk2"] = np.ascontiguousarray(bk[sl].reshape(P, 1))
            m["bvb"] = np.ascontiguousarray(
                np.broadcast_to(bv[sl], (P, P)), dtype=np.float32
            )
        maps.append(m)
    return maps


def _numpy_reference(query, key, value, mask, Wq, bq, Wk, bk, Wv, bv):
    """Pure-numpy fallback (only used when the mask isn't all ones)."""
    out = np.empty((B, S, DIM), dtype=np.float32)
    for b in range(B):
        q = (query[b] @ Wq.T + bq).reshape(S, NUM_HEADS, D_HEAD)
        k = (key[b] @ Wk.T + bk).reshape(S, NUM_HEADS, D_HEAD)
        v = (value[b] @ Wv.T + bv).reshape(S, NUM_HEADS, D_HEAD)
        for h in range(NUM_HEADS):
            s = q[:, h, :] @ k[:, h, :].T
            s = np.where(mask[b], s, np.float32(-10000.0))
            s = s / np.float32(np.sqrt(DIM))
            s = s - s.max(axis=-1, keepdims=True)
            e = np.exp(s)
            p = e / e.sum(axis=-1, keepdims=True)
            out[b, :, h * D_HEAD : (h + 1) * D_HEAD] = p @ v[:, h, :]
    return out


LAST_EXEC_NS = None
LAST_RESULTS = None


def kernel(query, key, value, mask, Wq, bq, Wk, bk, Wv, bv):
    global LAST_EXEC_NS, LAST_RESULTS
    query = np.asarray(query, dtype=np.float32)
    key = np.asarray(key, dtype=np.float32)
    value = np.asarray(value, dtype=np.float32)
    mask = np.asarray(mask)
    Wq = np.asarray(Wq, dtype=np.float32)
    bq = np.asarray(bq, dtype=np.float32)
    Wk = np.asarray(Wk, dtype=np.float32)
    bk = np.asarray(bk, dtype=np.float32)
    Wv = np.asarray(Wv, dtype=np.float32)
    bv = np.asarray(bv, dtype=np.float32)

    if not mask.all():
        return _numpy_reference(query, key, value, mask, Wq, bq, Wk, bk, Wv, bv)

    from concourse.bass_utils import run_bass_kernel_spmd

    has_bias = bool(bq.any() or bk.any() or bv.any())
    nc = _get_program(has_bias)
    in_maps = _shard_inputs(query, key, value, Wq, bq, Wk, bk, Wv, bv, has_bias)
    trace = os.environ.get("KERNEL_TRACE", "0") == "1"
    tmpdir = os.environ.get("KERNEL_TRACE_DIR") or None
    try:
        res = run_bass_kernel_spmd(
            nc, in_maps, list(range(N_CORES)), trace=trace, tmpdir=tmpdir
        )
    except Exception:
        if not trace:
            raise
        import traceback

        traceback.print_exc()
        res = run_bass_kernel_spmd(nc, in_maps, list(range(N_CORES)), trace=False)
    LAST_EXEC_NS = res.exec_time_ns
    LAST_RESULTS = res
    out = np.empty((B, S, DIM), dtype=np.float32)
    for c in range(N_CORES):
        b, g = c // 4, c % 4
        out[b, :, P * g : P * (g + 1)] = res.results[c]["out"]
    return out


# revision 20
# speedup vs baseline: 1.6486x; 1.2826x over previous
"""Multi-head attention (B=2, S=2048, DIM=512, H=8) on 8 Trainium2 cores.

Sharding: data-parallel over batch x tensor-parallel over heads.
Core c handles batch b = c // 4 and heads {2g, 2g+1} where g = c % 4
(i.e. output feature columns [128g : 128g+128]).  All sharding /
gathering happens host-side; no on-device collectives.

Per-core kernel. All matmul inputs are fp16 (10-bit mantissa, same
multiplier precision as TF32 for this N(0,1)-scaled data, but runs on
the fast normal PE path with fp32 PSUM accumulation); everything else
(PSUM, softmax denominators, normalization, output) stays fp32.

  - inputs arrive host-pretransposed as X^T [512, 2048] fp16; input
    DMAs are chunked by 512 seq columns so compute starts early,
  - Q^T, K^T projections in [out_dim(128), seq] layout (head h at
    partitions 64h..64h+63) - attention-ready; V in natural
    [seq, out_dim] tiles with a ones column per head so the ctx
    matmul also accumulates the softmax denominator for free,
  - scores^T = K_h @ Q_h^T per 128-row key tile (K=64), exp on
    ScalarE with the 1/sqrt(512) scale fused ([128,1024] tiles,
    fp16 output),
  - ctx^T accumulated over key tiles (lhsT = V tile [128,65],
    rhs = exp-scores [128,512], fp32 PSUM),
  - PE transpose back to natural layout, reciprocal + scale, DMA out.
"""

import os

import numpy as np

DIM = 512
NUM_HEADS = 8
D_HEAD = 64
B = 2
S = 2048
N_CORES = 8
P = 128  # partitions
NK = DIM // P  # 4 contraction tiles for projections
NT = S // P  # 16 key tiles
VSTRIDE = 132  # V tile stride: [h0(64) | ones | h1(64) | 3 pad]
SCALE = float(1.0 / np.sqrt(512.0))
CH = 512  # input DMA / projection chunk (columns of seq)

_CACHE = {}


def _build_program():
    import concourse.tile as tile
    from concourse import bacc, mybir

    f32 = mybir.dt.float32
    f16 = mybir.dt.float16
    nc = bacc.Bacc("TRN2", target_bir_lowering=False, debug=False)

    io = {}
    io["xqT"] = nc.dram_tensor("xqT", [DIM, S], f16, kind="ExternalInput").ap()
    io["xkT"] = nc.dram_tensor("xkT", [DIM, S], f16, kind="ExternalInput").ap()
    io["xvT"] = nc.dram_tensor("xvT", [DIM, S], f16, kind="ExternalInput").ap()
    io["wq"] = nc.dram_tensor("wq", [P, DIM], f16, kind="ExternalInput").ap()
    io["wk"] = nc.dram_tensor("wk", [P, DIM], f16, kind="ExternalInput").ap()
    io["wv"] = nc.dram_tensor("wv", [P, DIM], f16, kind="ExternalInput").ap()
    io["bq2"] = nc.dram_tensor("bq2", [P, 1], f32, kind="ExternalInput").ap()
    io["bk2"] = nc.dram_tensor("bk2", [P, 1], f32, kind="ExternalInput").ap()
    io["bvb"] = nc.dram_tensor("bvb", [P, P], f32, kind="ExternalInput").ap()
    io["ident"] = nc.dram_tensor("ident", [P, P], f32, kind="ExternalInput").ap()
    io["out"] = nc.dram_tensor("out", [S, P], f32, kind="ExternalOutput").ap()

    with tile.TileContext(nc) as tc:
        _emit(tc, mybir, io)
    nc.compile()
    return nc


def _emit(tc, mybir, io):
    from contextlib import ExitStack

    nc = tc.nc
    f32 = mybir.dt.float32
    f16 = mybir.dt.float16
    Exp = mybir.ActivationFunctionType.Exp

    mm = nc.tensor.matmul

    with ExitStack() as ctx:
        const = ctx.enter_context(tc.tile_pool(name="const", bufs=1))
        qk = ctx.enter_context(tc.tile_pool(name="qk", bufs=1))
        vpool = ctx.enter_context(tc.tile_pool(name="vpool", bufs=1))
        csbpool = ctx.enter_context(tc.tile_pool(name="csbp", bufs=2))
        osmall = ctx.enter_context(tc.tile_pool(name="osmall", bufs=4))

        # constants
        wq_sb = const.tile([P, DIM], f16, tag="wq")
        wk_sb = const.tile([P, DIM], f16, tag="wk")
        wv_sb = const.tile([P, DIM], f16, tag="wv")
        bq_sb = const.tile([P, 1], f32, tag="bq")
        bk_sb = const.tile([P, 1], f32, tag="bk")
        bvb_sb = const.tile([P, P], f32, tag="bvb")
        id_sb = const.tile([P, P], f32, tag="ident")
        nc.sync.dma_start(wq_sb[:], io["wq"][:])
        nc.sync.dma_start(wk_sb[:], io["wk"][:])
        nc.sync.dma_start(wv_sb[:], io["wv"][:])
        nc.sync.dma_start(bq_sb[:], io["bq2"][:])
        nc.sync.dma_start(bk_sb[:], io["bk2"][:])
        nc.sync.dma_start(bvb_sb[:], io["bvb"][:])
        nc.sync.dma_start(id_sb[:], io["ident"][:])

        # persistent projection outputs
        QT = qk.tile([P, S], f16, tag="QT")  # [out_dim, seq]
        KT = qk.tile([P, S], f16, tag="KT")
        V = vpool.tile([P, NT * VSTRIDE], f16, tag="V")  # 16 x [128, 132]

        with (
            tc.tile_pool(name="xin", bufs=24) as xin,
            tc.tile_pool(name="psq", bufs=2, space="PSUM") as psq,
            tc.tile_pool(name="psv", bufs=2, space="PSUM") as psv,
        ):
            for c in range(S // CH):
                cs = slice(c * CH, (c + 1) * CH)
                xt = {}
                for name, key in (("q", "xqT"), ("k", "xkT"), ("v", "xvT")):
                    ts = []
                    for k in range(NK):
                        tl = xin.tile([P, CH], f16, tag="xt")
                        nc.sync.dma_start(tl[:], io[key][k * P : (k + 1) * P, cs])
                        ts.append(tl)
                    xt[name] = ts

                for name, w_sb, b_sb, dst in (
                    ("q", wq_sb, bq_sb, QT),
                    ("k", wk_sb, bk_sb, KT),
                ):
                    ps = psq.tile([P, CH], f32, tag="psq")
                    for k in range(NK):
                        mm(
                            ps[:],
                            w_sb[:, k * P : (k + 1) * P],
                            xt[name][k][:, :],
                            start=(k == 0),
                            stop=(k == NK - 1),
                        )
                    nc.vector.tensor_scalar_add(dst[:, cs], ps[:], b_sb[:, 0:1])

                # V natural-layout tiles for this chunk (+ones column)
                for tl_i in range(CH // P):
                    ti = c * (CH // P) + tl_i
                    lsl = slice(tl_i * P, (tl_i + 1) * P)
                    ps = psv.tile([P, P], f32, tag="psv")
                    for k in range(NK):
                        mm(
                            ps[:],
                            xt["v"][k][:, lsl],
                            wv_sb[:, k * P : (k + 1) * P],
                            start=(k == 0),
                            stop=(k == NK - 1),
                        )
                    o = ti * VSTRIDE
                    nc.vector.memset(V[:, o + 64 : o + 65], 1.0)
                    nc.vector.tensor_add(
                        V[:, o : o + 64], ps[:, 0:64], bvb_sb[:, 0:64]
                    )
                    nc.vector.tensor_add(
                        V[:, o + 65 : o + 129], ps[:, 64:128], bvb_sb[:, 64:128]
                    )

        # ---- attention (+ inline transpose/normalize/store) ----
        with (
            tc.tile_pool(name="pss", bufs=2, space="PSUM") as pss,
            tc.tile_pool(name="psc", bufs=1, space="PSUM") as psc,
            tc.tile_pool(name="pst", bufs=2, space="PSUM") as pst,
            tc.tile_pool(name="es", bufs=3) as espool,
        ):
            for h in range(2):
                hp = 64 * h
                sumcol = 64 if h == 0 else 0
                for half in range(2):
                    q0 = 1024 * half
                    cps = psc.tile([65, 1024], f32, tag="cps")
                    for t in range(NT):
                        sps = pss.tile([P, 1024], f32, tag="sps")
                        for cc in range(2):
                            mm(
                                sps[:, cc * 512 : (cc + 1) * 512],
                                KT[hp : hp + 64, t * P : (t + 1) * P],
                                QT[hp : hp + 64, q0 + cc * 512 : q0 + (cc + 1) * 512],
                                start=True,
                                stop=True,
                            )
                        es = espool.tile([P, 1024], f16, tag="es")
                        nc.scalar.activation(es[:], sps[:], Exp, scale=SCALE)
                        vo = t * VSTRIDE + (64 if h == 1 else 0)
                        for cc in range(2):
                            mm(
                                cps[:, cc * 512 : (cc + 1) * 512],
                                V[:, vo : vo + 65],
                                es[:, cc * 512 : (cc + 1) * 512],
                                start=(t == 0),
                                stop=(t == NT - 1),
                            )
                    # evacuate ctx^T, transpose back, normalize, store
                    csb = csbpool.tile([65, 1024], f32, tag="csb")
                    nc.vector.tensor_copy(csb[:], cps[:])
                    for u in range(8):
                        tp = pst.tile([P, 65], f32, tag="tp")
                        nc.tensor.transpose(
                            tp[:], csb[:, u * P : (u + 1) * P], id_sb[0:65, 0:65]
                        )
                        r = osmall.tile([P, 1], f32, tag="recip")
                        nc.vector.reciprocal(r[:], tp[:, sumcol : sumcol + 1])
                        o = osmall.tile([P, 64], f32, tag="o")
                        if h == 0:
                            nc.vector.tensor_scalar_mul(o[:], tp[:, 0:64], r[:, 0:1])
                        else:
                            nc.vector.tensor_scalar_mul(o[:], tp[:, 1:65], r[:, 0:1])
                        nc.sync.dma_start(
                            io["out"][q0 + u * P : q0 + (u + 1) * P, hp : hp + 64],
                            o[:],
                        )


def _get_program():
    if "nc" not in _CACHE:
        _CACHE["nc"] = _build_program()
    return _CACHE["nc"]


def _shard_inputs(query, key, value, Wq, bq, Wk, bk, Wv, bv):
    """Build the 8 per-core input dicts (x and W as fp16)."""
    ident = np.eye(P, dtype=np.float32)
    maps = []
    xT = {}
    for b in range(B):
        xT[b] = (
            np.ascontiguousarray(query[b].T.astype(np.float16)),
            np.ascontiguousarray(key[b].T.astype(np.float16)),
            np.ascontiguousarray(value[b].T.astype(np.float16)),
        )

    def wslice(W, g):
        # want w[p, 128k + m] = W[128g + m, 128k + p]
        Ws = W[P * g : P * (g + 1), :]  # [m, 512]
        return np.ascontiguousarray(
            Ws.reshape(P, NK, P).transpose(2, 1, 0).reshape(P, DIM).astype(np.float16)
        )

    for c in range(N_CORES):
        b, g = c // 4, c % 4
        sl = slice(P * g, P * (g + 1))
        maps.append(
            {
                "xqT": xT[b][0],
                "xkT": xT[b][1],
                "xvT": xT[b][2],
                "wq": wslice(Wq, g),
                "wk": wslice(Wk, g),
                "wv": wslice(Wv, g),
                "bq2": np.ascontiguousarray(bq[sl].reshape(P, 1), dtype=np.float32),
                "bk2": np.ascontiguousarray(bk[sl].reshape(P, 1), dtype=np.float32),
                "bvb": np.ascontiguousarray(
                    np.broadcast_to(bv[sl], (P, P)), dtype=np.float32
                ),
                "ident": ident,
            }
        )
    return maps


def _numpy_reference(query, key, value, mask, Wq, bq, Wk, bk, Wv, bv):
    """Pure-numpy fallback (only used when the mask isn't all ones)."""
    out = np.empty((B, S, DIM), dtype=np.float32)
    for b in range(B):
        q = (query[b] @ Wq.T + bq).reshape(S, NUM_HEADS, D_HEAD)
        k = (key[b] @ Wk.T + bk).reshape(S, NUM_HEADS, D_HEAD)
        v = (value[b] @ Wv.T + bv).reshape(S, NUM_HEADS, D_HEAD)
        for h in range(NUM_HEADS):
            s = q[:, h, :] @ k[:, h, :].T
            s = np.where(mask[b], s, np.float32(-10000.0))
            s = s / np.float32(np.sqrt(DIM))
            s = s - s.max(axis=-1, keepdims=True)
            e = np.exp(s)
            p = e / e.sum(axis=-1, keepdims=True)
            out[b, :, h * D_HEAD : (h + 1) * D_HEAD] = p @ v[:, h, :]
    return out


LAST_EXEC_NS = None
LAST_RESULTS = None


def kernel(query, key, value, mask, Wq, bq, Wk, bk, Wv, bv):
    global LAST_EXEC_NS, LAST_RESULTS
    query = np.asarray(query, dtype=np.float32)
    key = np.asarray(key, dtype=np.float32)
    value = np.asarray(value, dtype=np.float32)
    mask = np.asarray(mask)
    Wq = np.asarray(Wq, dtype=np.float32)
    bq = np.asarray(bq, dtype=np.float32)
    Wk = np.asarray(Wk, dtype=np.float32)
    bk = np.asarray(bk, dtype=np.float32)
    Wv = np.asarray(Wv, dtype=np.float32)
    bv = np.asarray(bv, dtype=np.float32)

    if not mask.all():
        return _numpy_reference(query, key, value, mask, Wq, bq, Wk, bk, Wv, bv)

    from concourse.bass_utils import run_bass_kernel_spmd

    nc = _get_program()
    in_maps = _shard_inputs(query, key, value, Wq, bq, Wk, bk, Wv, bv)
    trace = os.environ.get("KERNEL_TRACE", "0") == "1"
    tmpdir = os.environ.get("KERNEL_TRACE_DIR") or None
    try:
        res = run_bass_kernel_spmd(
            nc, in_maps, list(range(N_CORES)), trace=trace, tmpdir=tmpdir
        )
    except Exception:
        if not trace:
            raise
        import traceback

        traceback.print_exc()
        res = run_bass_kernel_spmd(nc, in_maps, list(range(N_CORES)), trace=False)
    LAST_EXEC_NS = res.exec_time_ns
    LAST_RESULTS = res
    out = np.empty((B, S, DIM), dtype=np.float32)
    for c in range(N_CORES):
        b, g = c // 4, c % 4
        out[b, :, P * g : P * (g + 1)] = res.results[c]["out"]
    return out


# revision 21
# speedup vs baseline: 1.6534x; 1.0029x over previous
"""Multi-head attention (B=2, S=2048, DIM=512, H=8) on 8 Trainium2 cores.

Sharding: data-parallel over batch x tensor-parallel over heads.
Core c handles batch b = c // 4 and heads {2g, 2g+1} where g = c % 4
(i.e. output feature columns [128g : 128g+128]).  All sharding /
gathering happens host-side; no on-device collectives.

Per-core kernel. All matmul inputs are fp16 (10-bit mantissa, same
multiplier precision as TF32 for this N(0,1)-scaled data, but runs on
the fast normal PE path with fp32 PSUM accumulation); everything else
(PSUM, softmax denominators, normalization, output) stays fp32.

  - inputs arrive host-pretransposed as X^T [512, 2048] fp16; input
    DMAs are chunked by 512 seq columns so compute starts early,
  - Q^T, K^T projections in [out_dim(128), seq] layout (head h at
    partitions 64h..64h+63) - attention-ready; V in natural
    [seq, out_dim] tiles with a ones column per head so the ctx
    matmul also accumulates the softmax denominator for free,
  - scores^T = K_h @ Q_h^T per 128-row key tile (K=64), exp on
    ScalarE with the 1/sqrt(512) scale fused ([128,1024] tiles,
    fp16 output),
  - ctx^T accumulated over key tiles (lhsT = V tile [128,65],
    rhs = exp-scores [128,512], fp32 PSUM),
  - PE transpose back to natural layout, reciprocal + scale, DMA out.
"""

import os

import ml_dtypes
import numpy as np

DIM = 512
NUM_HEADS = 8
D_HEAD = 64
B = 2
S = 2048
N_CORES = 8
P = 128  # partitions
NK = DIM // P  # 4 contraction tiles for projections
NT = S // P  # 16 key tiles
VSTRIDE = 132  # V tile stride: [h0(64) | ones | h1(64) | 3 pad]
SCALE = float(1.0 / np.sqrt(512.0))
CH = 512  # input DMA / projection chunk (columns of seq)

_CACHE = {}


def _build_program():
    import concourse.tile as tile
    from concourse import bacc, mybir

    f32 = mybir.dt.float32
    f16 = mybir.dt.bfloat16
    nc = bacc.Bacc("TRN2", target_bir_lowering=False, debug=False)

    io = {}
    io["xqT"] = nc.dram_tensor("xqT", [DIM, S], f16, kind="ExternalInput").ap()
    io["xkT"] = nc.dram_tensor("xkT", [DIM, S], f16, kind="ExternalInput").ap()
    io["xvT"] = nc.dram_tensor("xvT", [DIM, S], f16, kind="ExternalInput").ap()
    io["wq"] = nc.dram_tensor("wq", [P, DIM], f16, kind="ExternalInput").ap()
    io["wk"] = nc.dram_tensor("wk", [P, DIM], f16, kind="ExternalInput").ap()
    io["wv"] = nc.dram_tensor("wv", [P, DIM], f16, kind="ExternalInput").ap()
    io["bq2"] = nc.dram_tensor("bq2", [P, 1], f32, kind="ExternalInput").ap()
    io["bk2"] = nc.dram_tensor("bk2", [P, 1], f32, kind="ExternalInput").ap()
    io["bvb"] = nc.dram_tensor("bvb", [P, P], f32, kind="ExternalInput").ap()
    io["ident"] = nc.dram_tensor("ident", [P, P], f32, kind="ExternalInput").ap()
    io["out"] = nc.dram_tensor("out", [S, P], f32, kind="ExternalOutput").ap()

    with tile.TileContext(nc) as tc:
        _emit(tc, mybir, io)
    nc.compile()
    return nc


def _emit(tc, mybir, io):
    from contextlib import ExitStack

    nc = tc.nc
    f32 = mybir.dt.float32
    f16 = mybir.dt.bfloat16
    Exp = mybir.ActivationFunctionType.Exp

    mm = nc.tensor.matmul

    with ExitStack() as ctx:
        const = ctx.enter_context(tc.tile_pool(name="const", bufs=1))
        qk = ctx.enter_context(tc.tile_pool(name="qk", bufs=1))
        vpool = ctx.enter_context(tc.tile_pool(name="vpool", bufs=1))
        csbpool = ctx.enter_context(tc.tile_pool(name="csbp", bufs=2))
        osmall = ctx.enter_context(tc.tile_pool(name="osmall", bufs=4))

        # constants
        wq_sb = const.tile([P, DIM], f16, tag="wq")
        wk_sb = const.tile([P, DIM], f16, tag="wk")
        wv_sb = const.tile([P, DIM], f16, tag="wv")
        bq_sb = const.tile([P, 1], f32, tag="bq")
        bk_sb = const.tile([P, 1], f32, tag="bk")
        bvb_sb = const.tile([P, P], f32, tag="bvb")
        id_sb = const.tile([P, P], f32, tag="ident")
        nc.sync.dma_start(wq_sb[:], io["wq"][:])
        nc.sync.dma_start(wk_sb[:], io["wk"][:])
        nc.sync.dma_start(wv_sb[:], io["wv"][:])
        nc.sync.dma_start(bq_sb[:], io["bq2"][:])
        nc.sync.dma_start(bk_sb[:], io["bk2"][:])
        nc.sync.dma_start(bvb_sb[:], io["bvb"][:])
        nc.sync.dma_start(id_sb[:], io["ident"][:])

        # persistent projection outputs
        QT = qk.tile([P, S], f16, tag="QT")  # [out_dim, seq]
        KT = qk.tile([P, S], f16, tag="KT")
        V = vpool.tile([P, NT * VSTRIDE], f16, tag="V")  # 16 x [128, 132]

        with (
            tc.tile_pool(name="xin", bufs=24) as xin,
            tc.tile_pool(name="psq", bufs=2, space="PSUM") as psq,
            tc.tile_pool(name="psv", bufs=2, space="PSUM") as psv,
        ):
            for c in range(S // CH):
                cs = slice(c * CH, (c + 1) * CH)
                xt = {}
                for name, key in (("q", "xqT"), ("k", "xkT"), ("v", "xvT")):
                    ts = []
                    for k in range(NK):
                        tl = xin.tile([P, CH], f16, tag="xt")
                        nc.sync.dma_start(tl[:], io[key][k * P : (k + 1) * P, cs])
                        ts.append(tl)
                    xt[name] = ts

                for name, w_sb, b_sb, dst in (
                    ("q", wq_sb, bq_sb, QT),
                    ("k", wk_sb, bk_sb, KT),
                ):
                    ps = psq.tile([P, CH], f32, tag="psq")
                    for k in range(NK):
                        mm(
                            ps[:],
                            w_sb[:, k * P : (k + 1) * P],
                            xt[name][k][:, :],
                            start=(k == 0),
                            stop=(k == NK - 1),
                        )
                    nc.vector.tensor_scalar_add(dst[:, cs], ps[:], b_sb[:, 0:1])

                # V natural-layout tiles for this chunk (+ones column)
                for tl_i in range(CH // P):
                    ti = c * (CH // P) + tl_i
                    lsl = slice(tl_i * P, (tl_i + 1) * P)
                    ps = psv.tile([P, P], f32, tag="psv")
                    for k in range(NK):
                        mm(
                            ps[:],
                            xt["v"][k][:, lsl],
                            wv_sb[:, k * P : (k + 1) * P],
                            start=(k == 0),
                            stop=(k == NK - 1),
                        )
                    o = ti * VSTRIDE
                    nc.vector.memset(V[:, o + 64 : o + 65], 1.0)
                    nc.vector.tensor_add(
                        V[:, o : o + 64], ps[:, 0:64], bvb_sb[:, 0:64]
                    )
                    nc.vector.tensor_add(
                        V[:, o + 65 : o + 129], ps[:, 64:128], bvb_sb[:, 64:128]
                    )

        # ---- attention (+ inline transpose/normalize/store) ----
        with (
            tc.tile_pool(name="pss", bufs=2, space="PSUM") as pss,
            tc.tile_pool(name="psc", bufs=1, space="PSUM") as psc,
            tc.tile_pool(name="pst", bufs=2, space="PSUM") as pst,
            tc.tile_pool(name="es", bufs=3) as espool,
        ):
            for h in range(2):
                hp = 64 * h
                sumcol = 64 if h == 0 else 0
                for half in range(2):
                    q0 = 1024 * half
                    cps = psc.tile([65, 1024], f32, tag="cps")
                    for t in range(NT):
                        sps = pss.tile([P, 1024], f32, tag="sps")
                        for cc in range(2):
                            mm(
                                sps[:, cc * 512 : (cc + 1) * 512],
                                KT[hp : hp + 64, t * P : (t + 1) * P],
                                QT[hp : hp + 64, q0 + cc * 512 : q0 + (cc + 1) * 512],
                                start=True,
                                stop=True,
                            )
                        es = espool.tile([P, 1024], f16, tag="es")
                        nc.scalar.activation(es[:], sps[:], Exp, scale=SCALE)
                        vo = t * VSTRIDE + (64 if h == 1 else 0)
                        for cc in range(2):
                            mm(
                                cps[:, cc * 512 : (cc + 1) * 512],
                                V[:, vo : vo + 65],
                                es[:, cc * 512 : (cc + 1) * 512],
                                start=(t == 0),
                                stop=(t == NT - 1),
                            )
                    # evacuate ctx^T, transpose back, normalize, store
                    csb = csbpool.tile([65, 1024], f32, tag="csb")
                    nc.vector.tensor_copy(csb[:], cps[:])
                    for u in range(8):
                        tp = pst.tile([P, 65], f32, tag="tp")
                        nc.tensor.transpose(
                            tp[:], csb[:, u * P : (u + 1) * P], id_sb[0:65, 0:65]
                        )
                        r = osmall.tile([P, 1], f32, tag="recip")
                        nc.vector.reciprocal(r[:], tp[:, sumcol : sumcol + 1])
                        o = osmall.tile([P, 64], f32, tag="o")
                        if h == 0:
                            nc.vector.tensor_scalar_mul(o[:], tp[:, 0:64], r[:, 0:1])
                        else:
                            nc.vector.tensor_scalar_mul(o[:], tp[:, 1:65], r[:, 0:1])
                        nc.sync.dma_start(
                            io["out"][q0 + u * P : q0 + (u + 1) * P, hp : hp + 64],
                            o[:],
                        )


def _get_program():
    if "nc" not in _CACHE:
        _CACHE["nc"] = _build_program()
    return _CACHE["nc"]


def _shard_inputs(query, key, value, Wq, bq, Wk, bk, Wv, bv):
    """Build the 8 per-core input dicts (x and W as fp16)."""
    ident = np.eye(P, dtype=np.float32)
    maps = []
    xT = {}
    for b in range(B):
        xT[b] = (
            np.ascontiguousarray(query[b].T.astype(ml_dtypes.bfloat16)),
            np.ascontiguousarray(key[b].T.astype(ml_dtypes.bfloat16)),
            np.ascontiguousarray(value[b].T.astype(ml_dtypes.bfloat16)),
        )

    def wslice(W, g):
        # want w[p, 128k + m] = W[128g + m, 128k + p]
        Ws = W[P * g : P * (g + 1), :]  # [m, 512]
        return np.ascontiguousarray(
            Ws.reshape(P, NK, P).transpose(2, 1, 0).reshape(P, DIM).astype(ml_dtypes.bfloat16)
        )

    for c in range(N_CORES):
        b, g = c // 4, c % 4
        sl = slice(P * g, P * (g + 1))
        maps.append(
            {
                "xqT": xT[b][0],
                "xkT": xT[b][1],
                "xvT": xT[b][2],
                "wq": wslice(Wq, g),
                "wk": wslice(Wk, g),
                "wv": wslice(Wv, g),
                "bq2": np.ascontiguousarray(bq[sl].reshape(P, 1), dtype=np.float32),
                "bk2": np.ascontiguousarray(bk[sl].reshape(P, 1), dtype=np.float32),
                "bvb": np.ascontiguousarray(
                    np.broadcast_to(bv[sl], (P, P)), dtype=np.float32
                ),
                "ident": ident,
            }
        )
    return maps


def _numpy_reference(query, key, value, mask, Wq, bq, Wk, bk, Wv, bv):
    """Pure-numpy fallback (only used when the mask isn't all ones)."""
    out = np.empty((B, S, DIM), dtype=np.float32)
    for b in range(B):
        q = (query[b] @ Wq.T + bq).reshape(S, NUM_HEADS, D_HEAD)
        k = (key[b] @ Wk.T + bk).reshape(S, NUM_HEADS, D_HEAD)
        v = (value[b] @ Wv.T + bv).reshape(S, NUM_HEADS, D_HEAD)
        for h in range(NUM_HEADS):
            s = q[:, h, :] @ k[:, h, :].T
            s = np.where(mask[b], s, np.float32(-10000.0))
            s = s / np.float32(np.sqrt(DIM))
            s = s - s.max(axis=-1, keepdims=True)
            e = np.exp(s)
            p = e / e.sum(axis=-1, keepdims=True)
            out[b, :, h * D_HEAD : (h + 1) * D_HEAD] = p @ v[:, h, :]
    return out


LAST_EXEC_NS = None
LAST_RESULTS = None


def kernel(query, key, value, mask, Wq, bq, Wk, bk, Wv, bv):
    global LAST_EXEC_NS, LAST_RESULTS
    query = np.asarray(query, dtype=np.float32)
    key = np.asarray(key, dtype=np.float32)
    value = np.asarray(value, dtype=np.float32)
    mask = np.asarray(mask)
    Wq = np.asarray(Wq, dtype=np.float32)
    bq = np.asarray(bq, dtype=np.float32)
    Wk = np.asarray(Wk, dtype=np.float32)
    bk = np.asarray(bk, dtype=np.float32)
    Wv = np.asarray(Wv, dtype=np.float32)
    bv = np.asarray(bv, dtype=np.float32)

    if not mask.all():
        return _numpy_reference(query, key, value, mask, Wq, bq, Wk, bk, Wv, bv)

    from concourse.bass_utils import run_bass_kernel_spmd

    nc = _get_program()
    in_maps = _shard_inputs(query, key, value, Wq, bq, Wk, bk, Wv, bv)
    trace = os.environ.get("KERNEL_TRACE", "0") == "1"
    tmpdir = os.environ.get("KERNEL_TRACE_DIR") or None
    try:
        res = run_bass_kernel_spmd(
            nc, in_maps, list(range(N_CORES)), trace=trace, tmpdir=tmpdir
        )
    except Exception:
        if not trace:
            raise
        import traceback

        traceback.print_exc()
        res = run_bass_kernel_spmd(nc, in_maps, list(range(N_CORES)), trace=False)
    LAST_EXEC_NS = res.exec_time_ns
    LAST_RESULTS = res
    out = np.empty((B, S, DIM), dtype=np.float32)
    for c in range(N_CORES):
        b, g = c // 4, c % 4
        out[b, :, P * g : P * (g + 1)] = res.results[c]["out"]
    return out


# revision 23
# speedup vs baseline: 2.3290x; 1.4086x over previous
"""Multi-head attention (B=2, S=2048, DIM=512, H=8) on 8 Trainium2 cores.

Sharding: data-parallel over batch x tensor-parallel over heads.
Core c handles batch b = c // 4 and heads {2g, 2g+1} where g = c % 4
(i.e. output feature columns [128g : 128g+128]).  All sharding /
gathering happens host-side; no on-device collectives.

Per-core kernel. All matmul inputs are fp16 (10-bit mantissa, same
multiplier precision as TF32 for this N(0,1)-scaled data, but runs on
the fast normal PE path with fp32 PSUM accumulation); everything else
(PSUM, softmax denominators, normalization, output) stays fp32.

  - inputs arrive host-pretransposed as X^T [512, 2048] fp16; input
    DMAs are chunked by 512 seq columns so compute starts early,
  - Q^T, K^T projections in [out_dim(128), seq] layout (head h at
    partitions 64h..64h+63) - attention-ready; V in natural
    [seq, out_dim] tiles with a ones column per head so the ctx
    matmul also accumulates the softmax denominator for free,
  - scores^T = K_h @ Q_h^T per 128-row key tile (K=64), exp on
    ScalarE with the 1/sqrt(512) scale fused ([128,1024] tiles,
    fp16 output),
  - ctx^T accumulated over key tiles (lhsT = V tile [128,65],
    rhs = exp-scores [128,512], fp32 PSUM),
  - PE transpose back to natural layout, reciprocal + scale, DMA out.
"""

import os

import ml_dtypes
import numpy as np

DIM = 512
NUM_HEADS = 8
D_HEAD = 64
B = 2
S = 2048
N_CORES = 8
P = 128  # partitions
NK = DIM // P  # 4 contraction tiles for projections
NT = S // P  # 16 key tiles
VSTRIDE = 132  # V tile stride: [h0(64) | ones | h1(64) | 3 pad]
SCALE = float(1.0 / np.sqrt(512.0))
CH = 512  # input DMA / projection chunk (columns of seq)

_CACHE = {}


def _build_program():
    import concourse.tile as tile
    from concourse import bacc, mybir

    f32 = mybir.dt.float32
    f16 = mybir.dt.float16
    nc = bacc.Bacc("TRN2", target_bir_lowering=False, debug=False)

    io = {}
    io["xqT"] = nc.dram_tensor("xqT", [DIM, S], f16, kind="ExternalInput").ap()
    io["xkT"] = nc.dram_tensor("xkT", [DIM, S], f16, kind="ExternalInput").ap()
    io["xvT"] = nc.dram_tensor("xvT", [DIM, S], f16, kind="ExternalInput").ap()
    io["wq"] = nc.dram_tensor("wq", [P, DIM], f16, kind="ExternalInput").ap()
    io["wk"] = nc.dram_tensor("wk", [P, DIM], f16, kind="ExternalInput").ap()
    io["wv"] = nc.dram_tensor("wv", [P, DIM], f16, kind="ExternalInput").ap()
    io["bq2"] = nc.dram_tensor("bq2", [P, 1], f32, kind="ExternalInput").ap()
    io["bk2"] = nc.dram_tensor("bk2", [P, 1], f32, kind="ExternalInput").ap()
    io["bvb"] = nc.dram_tensor("bvb", [P, P], f32, kind="ExternalInput").ap()
    io["ident"] = nc.dram_tensor("ident", [P, P], f32, kind="ExternalInput").ap()
    io["out"] = nc.dram_tensor("out", [S, P], f32, kind="ExternalOutput").ap()

    with tile.TileContext(nc) as tc:
        _emit(tc, mybir, io)
    nc.compile()
    return nc


def _emit(tc, mybir, io):
    from contextlib import ExitStack

    nc = tc.nc
    f32 = mybir.dt.float32
    f16 = mybir.dt.float16
    Exp = mybir.ActivationFunctionType.Exp

    mm = nc.tensor.matmul

    with ExitStack() as ctx:
        const = ctx.enter_context(tc.tile_pool(name="const", bufs=1))
        qk = ctx.enter_context(tc.tile_pool(name="qk", bufs=1))
        vpool = ctx.enter_context(tc.tile_pool(name="vpool", bufs=1))
        csbpool = ctx.enter_context(tc.tile_pool(name="csbp", bufs=2))
        osmall = ctx.enter_context(tc.tile_pool(name="osmall", bufs=4))

        # constants
        wq_sb = const.tile([P, DIM], f16, tag="wq")
        wk_sb = const.tile([P, DIM], f16, tag="wk")
        wv_sb = const.tile([P, DIM], f16, tag="wv")
        bq_sb = const.tile([P, 1], f32, tag="bq")
        bk_sb = const.tile([P, 1], f32, tag="bk")
        bvb_sb = const.tile([P, P], f32, tag="bvb")
        id_sb = const.tile([P, P], f32, tag="ident")
        nc.sync.dma_start(wq_sb[:], io["wq"][:])
        nc.sync.dma_start(wk_sb[:], io["wk"][:])
        nc.sync.dma_start(wv_sb[:], io["wv"][:])
        nc.sync.dma_start(bq_sb[:], io["bq2"][:])
        nc.sync.dma_start(bk_sb[:], io["bk2"][:])
        nc.sync.dma_start(bvb_sb[:], io["bvb"][:])
        nc.sync.dma_start(id_sb[:], io["ident"][:])

        # persistent projection outputs
        QT = qk.tile([P, S], f16, tag="QT")  # [out_dim, seq]
        KT = qk.tile([P, S], f16, tag="KT")
        V = vpool.tile([P, NT * VSTRIDE], f16, tag="V")  # 16 x [128, 132]

        with (
            tc.tile_pool(name="xin", bufs=24) as xin,
            tc.tile_pool(name="psq", bufs=2, space="PSUM") as psq,
            tc.tile_pool(name="psv", bufs=2, space="PSUM") as psv,
        ):
            for c in range(S // CH):
                cs = slice(c * CH, (c + 1) * CH)
                xt = {}
                for name, key in (("q", "xqT"), ("k", "xkT"), ("v", "xvT")):
                    ts = []
                    for k in range(NK):
                        tl = xin.tile([P, CH], f16, tag="xt")
                        nc.sync.dma_start(tl[:], io[key][k * P : (k + 1) * P, cs])
                        ts.append(tl)
                    xt[name] = ts

                for name, w_sb, b_sb, dst in (
                    ("q", wq_sb, bq_sb, QT),
                    ("k", wk_sb, bk_sb, KT),
                ):
                    ps = psq.tile([P, CH], f32, tag="psq")
                    for k in range(NK):
                        mm(
                            ps[:],
                            w_sb[:, k * P : (k + 1) * P],
                            xt[name][k][:, :],
                            start=(k == 0),
                            stop=(k == NK - 1),
                        )
                    nc.vector.tensor_scalar_add(dst[:, cs], ps[:], b_sb[:, 0:1])

                # V natural-layout tiles for this chunk (+ones column)
                for tl_i in range(CH // P):
                    ti = c * (CH // P) + tl_i
                    lsl = slice(tl_i * P, (tl_i + 1) * P)
                    ps = psv.tile([P, P], f32, tag="psv")
                    for k in range(NK):
                        mm(
                            ps[:],
                            xt["v"][k][:, lsl],
                            wv_sb[:, k * P : (k + 1) * P],
                            start=(k == 0),
                            stop=(k == NK - 1),
                        )
                    o = ti * VSTRIDE
                    nc.vector.memset(V[:, o + 64 : o + 65], 1.0)
                    nc.vector.tensor_add(
                        V[:, o : o + 64], ps[:, 0:64], bvb_sb[:, 0:64]
                    )
                    nc.vector.tensor_add(
                        V[:, o + 65 : o + 129], ps[:, 64:128], bvb_sb[:, 64:128]
                    )

        # ---- attention (+ inline transpose/normalize/store) ----
        # Both heads per iteration: the two K=64 score matmuls target
        # disjoint PE row groups (partitions 0-63 / 64-127) so they run
        # concurrently AND register as full-array activity for the HAM
        # clock governor (unpacked K=64 streams never un-throttle the
        # PE).  ctx matmuls are K=128/M=65 which hold the warm clock.
        with (
            tc.tile_pool(name="pss", bufs=2, space="PSUM") as pss,
            tc.tile_pool(name="psc", bufs=1, space="PSUM") as psc,
            tc.tile_pool(name="pst", bufs=2, space="PSUM") as pst,
            tc.tile_pool(name="es", bufs=3) as espool,
        ):
            for q in range(4):  # 512-wide query chunks
                qs = slice(q * 512, (q + 1) * 512)
                cps = {
                    0: psc.tile([65, 512], f32, tag="c0", name="c0"),
                    1: psc.tile([65, 512], f32, tag="c1", name="c1"),
                }
                for t in range(NT):
                    sps = pss.tile([P, 1024], f32, tag="sps")
                    for h in range(2):
                        hp = 64 * h
                        mm(
                            sps[:, h * 512 : (h + 1) * 512],
                            KT[hp : hp + 64, t * P : (t + 1) * P],
                            QT[hp : hp + 64, qs],
                            start=True,
                            stop=True,
                        )
                    es = espool.tile([P, 1024], f16, tag="es")
                    nc.scalar.activation(es[:], sps[:], Exp, scale=SCALE)
                    for h in range(2):
                        vo = t * VSTRIDE + 64 * h
                        mm(
                            cps[h][:],
                            V[:, vo : vo + 65],
                            es[:, h * 512 : (h + 1) * 512],
                            start=(t == 0),
                            stop=(t == NT - 1),
                        )
                # evacuate ctx^T, transpose back, normalize, store
                for h in range(2):
                    hp = 64 * h
                    sumcol = 64 if h == 0 else 0
                    csb = csbpool.tile([65, 512], f32, tag="csb")
                    nc.vector.tensor_copy(csb[:], cps[h][:])
                    for u in range(4):
                        tp = pst.tile([P, 65], f32, tag="tp")
                        nc.tensor.transpose(
                            tp[:], csb[:, u * P : (u + 1) * P], id_sb[0:65, 0:65]
                        )
                        r = osmall.tile([P, 1], f32, tag="recip")
                        nc.vector.reciprocal(r[:], tp[:, sumcol : sumcol + 1])
                        o = osmall.tile([P, 64], f32, tag="o")
                        if h == 0:
                            nc.vector.tensor_scalar_mul(o[:], tp[:, 0:64], r[:, 0:1])
                        else:
                            nc.vector.tensor_scalar_mul(o[:], tp[:, 1:65], r[:, 0:1])
                        nc.sync.dma_start(
                            io["out"][
                                q * 512 + u * P : q * 512 + (u + 1) * P,
                                hp : hp + 64,
                            ],
                            o[:],
                        )


def _get_program():
    if "nc" not in _CACHE:
        _CACHE["nc"] = _build_program()
    return _CACHE["nc"]


def _shard_inputs(query, key, value, Wq, bq, Wk, bk, Wv, bv):
    """Build the 8 per-core input dicts (x and W as fp16)."""
    ident = np.eye(P, dtype=np.float32)
    maps = []
    xT = {}
    for b in range(B):
        xT[b] = (
            np.ascontiguousarray(query[b].T.astype(np.float16)),
            np.ascontiguousarray(key[b].T.astype(np.float16)),
            np.ascontiguousarray(value[b].T.astype(np.float16)),
        )

    def wslice(W, g):
        # want w[p, 128k + m] = W[128g + m, 128k + p]
        Ws = W[P * g : P * (g + 1), :]  # [m, 512]
        return np.ascontiguousarray(
            Ws.reshape(P, NK, P).transpose(2, 1, 0).reshape(P, DIM).astype(np.float16)
        )

    for c in range(N_CORES):
        b, g = c // 4, c % 4
        sl = slice(P * g, P * (g + 1))
        maps.append(
            {
                "xqT": xT[b][0],
                "xkT": xT[b][1],
                "xvT": xT[b][2],
                "wq": wslice(Wq, g),
                "wk": wslice(Wk, g),
                "wv": wslice(Wv, g),
                "bq2": np.ascontiguousarray(bq[sl].reshape(P, 1), dtype=np.float32),
                "bk2": np.ascontiguousarray(bk[sl].reshape(P, 1), dtype=np.float32),
                "bvb": np.ascontiguousarray(
                    np.broadcast_to(bv[sl], (P, P)), dtype=np.float32
                ),
                "ident": ident,
            }
        )
    return maps


def _numpy_reference(query, key, value, mask, Wq, bq, Wk, bk, Wv, bv):
    """Pure-numpy fallback (only used when the mask isn't all ones)."""
    out = np.empty((B, S, DIM), dtype=np.float32)
    for b in range(B):
        q = (query[b] @ Wq.T + bq).reshape(S, NUM_HEADS, D_HEAD)
        k = (key[b] @ Wk.T + bk).reshape(S, NUM_HEADS, D_HEAD)
        v = (value[b] @ Wv.T + bv).reshape(S, NUM_HEADS, D_HEAD)
        for h in range(NUM_HEADS):
            s = q[:, h, :] @ k[:, h, :].T
            s = np.where(mask[b], s, np.float32(-10000.0))
            s = s / np.float32(np.sqrt(DIM))
            s = s - s.max(axis=-1, keepdims=True)
            e = np.exp(s)
            p = e / e.sum(axis=-1, keepdims=True)
            out[b, :, h * D_HEAD : (h + 1) * D_HEAD] = p @ v[:, h, :]
    return out


LAST_EXEC_NS = None
LAST_RESULTS = None


def kernel(query, key, value, mask, Wq, bq, Wk, bk, Wv, bv):
    global LAST_EXEC_NS, LAST_RESULTS
    query = np.asarray(query, dtype=np.float32)
    key = np.asarray(key, dtype=np.float32)
    value = np.asarray(value, dtype=np.float32)
    mask = np.asarray(mask)
    Wq = np.asarray(Wq, dtype=np.float32)
    bq = np.asarray(bq, dtype=np.float32)
    Wk = np.asarray(Wk, dtype=np.float32)
    bk = np.asarray(bk, dtype=np.float32)
    Wv = np.asarray(Wv, dtype=np.float32)
    bv = np.asarray(bv, dtype=np.float32)

    if not mask.all():
        return _numpy_reference(query, key, value, mask, Wq, bq, Wk, bk, Wv, bv)

    from concourse.bass_utils import run_bass_kernel_spmd

    nc = _get_program()
    in_maps = _shard_inputs(query, key, value, Wq, bq, Wk, bk, Wv, bv)
    trace = os.environ.get("KERNEL_TRACE", "0") == "1"
    tmpdir = os.environ.get("KERNEL_TRACE_DIR") or None
    try:
        res = run_bass_kernel_spmd(
            nc, in_maps, list(range(N_CORES)), trace=trace, tmpdir=tmpdir
        )
    except Exception:
        if not trace:
            raise
        import traceback

        traceback.print_exc()
        res = run_bass_kernel_spmd(nc, in_maps, list(range(N_CORES)), trace=False)
    LAST_EXEC_NS = res.exec_time_ns
    LAST_RESULTS = res
    out = np.empty((B, S, DIM), dtype=np.float32)
    for c in range(N_CORES):
        b, g = c // 4, c % 4
        out[b, :, P * g : P * (g + 1)] = res.results[c]["out"]
    return out
